# revision 19
# baseline (speedup 1.0000x reference)
"""Trainium2 Bass kernel for CycleBalanceLoss.

loss = ALPHA * mean_b |sum_l adj[b, argmax_l, argmax_{l+1}]|
     + (1-ALPHA) * mean_{b,l} (logsumexp(logits[b,l,:]) - logits[b,l,t[b,l]])

Sharding: pure data parallel over the batch dim B=64 across 8 cores
(BPC=8 batches per core). Host sums the 8 per-core partial scalars.

Per core:
  - stream the logits shard [8, 128, 1024] f32 through SBUF;
  - ScalarE computes exp(x) with a per-row f32 accumulator (-> logsumexp)
    writing exp as fp16 so the DVE argmax (max/max_index) runs on 2-byte
    data (argmax(exp(x)) == argmax(x));
  - both gathers use Pool SWDGE dma_gather fetching the aligned 256B/512B
    block CONTAINING each needed element (vs. the old 16 serialized
    per-element DMA_INDIRECTs):
      * target logits: one dma_gather, indices host-precomputed in the
        wrapped int16 layout;
      * adjacency: indices depend on the device argmax. The idx[l+1]
        partition shift is a PE matmul with a shift matrix (zeroes the
        pad row for free); the wrapped-int16 fold+replicate is one
        broadcast-multiply + one PE matmul (W16REP). Three gather groups
        [0-3], [4-6], [7] (int16 block-index range caps a group at 4
        batches) so earlier groups overlap the batch loop and the tail
        only carries a 128-descriptor gather;
  - within-block selects are single fused scalar_tensor_tensor ops:
    (iota == rem[p]) * blocks with a sum accumulator;
  - tc.tile_wait_until phases pin queue order: the scheduler's SWDGE cost
    model is optimistic and otherwise hoists gather-dependent ops ahead
    of the argmax work, stalling the in-order DVE queue.
"""

import numpy as np

B, L, N = 64, 128, 1024
NCORES = 8
BPC = B // NCORES
ALPHA = 0.7

XE = 64    # xt gather block elems (256B)
AE = 64    # adj gather block elems (256B)
GROUPS = [(0, 2), (2, 4), (4, 6), (6, 8)]  # adj gather groups [lo, hi)
GQ = [0, 2, 3, 0]  # swdge queue per adj group (xt gather uses q1)

_CACHE = {}


def _build():
    import concourse.bacc as bacc
    import concourse.tile as tile
    from concourse import bass, library_config, mybir
    from concourse.bass import broadcast_tensor_aps

    f32 = mybir.dt.float32
    fp16 = mybir.dt.float16
    i16 = mybir.dt.int16
    u16 = mybir.dt.uint16
    AF = mybir.ActivationFunctionType
    Alu = mybir.AluOpType
    AX = mybir.AxisListType

    nc = bacc.Bacc(
        "TRN2",
        target_bir_lowering=False,
        debug=False,
        num_devices=NCORES,
        num_swdge_queues=4,
    )

    logits = nc.dram_tensor("logits", [BPC * L * N // XE, XE], f32, kind="ExternalInput")
    adj = nc.dram_tensor("adj", [BPC * N * N // AE, AE], f32, kind="ExternalInput")
    # cf: [0:8 xtrem | 8:16 kmat | 16:144 w16 | 144:208 iota | 208:336 shiftm | 336:344 ccf]
    cf = nc.dram_tensor("cf", [128, 344], f32, kind="ExternalInput")
    # cu: wrapped int16 xt block idxs
    cu = nc.dram_tensor("cu", [128, 64], i16, kind="ExternalInput")
    out = nc.dram_tensor("out", [2, 1], f32, kind="ExternalOutput")

    lg = logits.ap()
    av = adj.ap()
    ROWS_PER_B = N * N // AE  # adj view rows per batch

    with tile.TileContext(nc) as tc:
        with (
            tc.tile_pool(name="xp", bufs=4) as xp,
            tc.tile_pool(name="ep", bufs=3) as ep,
            tc.tile_pool(name="sp", bufs=2) as sp,
            tc.tile_pool(name="acc", bufs=1) as accp,
            tc.tile_pool(name="psum", bufs=1, space="PSUM") as pp,
        ):
            nc.gpsimd.load_library(library_config.mlp)

            CU = accp.tile([128, 64], i16)
            nc.scalar.dma_start(CU[:], cu.ap())
            CF = accp.tile([128, 344], f32)
            nc.scalar.dma_start(CF[:], cf.ap())
            XR = CF[:, 0:BPC]
            KM = CF[:, 8:16]
            WM = CF[:, 16:144]
            IO = CF[:, 144 : 144 + AE]
            SH = CF[:, 208:336]
            CCF = CF[:, 336:344]

            ones = accp.tile([L, 1], f32)
            nc.vector.memset(ones[:], 1.0)

            # target-logit blocks: gather starts as soon as CU lands
            XTB = accp.tile([128, BPC, XE], f32)
            nc.gpsimd.dma_gather(
                XTB[:], lg, CU[:], BPC * 128, BPC * 128, XE, queue_num=1
            )

            S = accp.tile([L, BPC], f32)
            IDXC = accp.tile([L, BPC], u16)  # argmax col per batch, contiguous
            M8 = accp.tile([L, BPC * 8], fp16)
            ADJB = accp.tile([128, BPC, AE], f32)
            AIDX = accp.tile([128, 64], i16)
            Wv = accp.tile([128, BPC], f32)
            XTv = accp.tile([128, BPC], f32)
            remf = accp.tile([L, BPC], f32)

            def batch(b):
                X = xp.tile([L, N], f32, tag="X")
                src = lg[b * 2048 : (b + 1) * 2048].rearrange(
                    "(l s) e -> l s e", l=L, s=N // XE
                )
                nc.sync.dma_start(X[:].rearrange("l (s e) -> l s e", s=N // XE), src)
                E = ep.tile([L, N], fp16, tag="E")
                nc.scalar.activation(E[:], X[:], AF.Exp, accum_out=S[:, b : b + 1])
                nc.vector.max(M8[:, 8 * b : 8 * b + 8], E[:])
                i8 = sp.tile([L, 8], u16, tag="i8")
                nc.vector.max_index(i8[:], M8[:, 8 * b : 8 * b + 8], E[:])
                nc.vector.tensor_copy(IDXC[:, b : b + 1], i8[:, 0:1])

            def idxcols(lo, hi):
                return IDXC[:, lo:hi]

            def adj_group(g):
                lo, hi = GROUPS[g]
                G = hi - lo
                cols = slice(lo, hi)
                # hi/lo parts of idx (natural layout), as f32 for the PE shift
                hl_u = sp.tile([L, 2 * G], u16, tag=f"hlu{g}")
                nc.vector.tensor_scalar(
                    hl_u[:, 0:G], idxcols(lo, hi), 6, None,
                    op0=Alu.logical_shift_right,
                )
                nc.vector.tensor_scalar(
                    hl_u[:, G : 2 * G], idxcols(lo, hi), AE - 1, None,
                    op0=Alu.bitwise_and,
                )
                hl_f = sp.tile([L, 2 * G], f32, tag=f"hlf{g}")
                nc.vector.tensor_copy(hl_f[:], hl_u[:])
                srcf = sp.tile([L, G], f32, tag=f"srcf{g}")
                nc.vector.tensor_copy(srcf[:], idxcols(lo, hi))
                # partition shift l -> l+1 via PE (row 127 becomes 0)
                shp = pp.tile([L, 2 * G], f32)
                nc.tensor.matmul(out=shp[:], lhsT=SH, rhs=hl_f[:], start=True, stop=True)
                nc.vector.tensor_copy(remf[:, cols], shp[:, G : 2 * G])
                # blk = src*8 + shifted_hi + cc
                blkf = sp.tile([L, G], f32, tag=f"blkf{g}")
                nc.vector.scalar_tensor_tensor(
                    blkf[:], srcf[:], 16.0, shp[:, 0:G], op0=Alu.mult, op1=Alu.add
                )
                nc.vector.tensor_tensor(blkf[:], blkf[:], CCF[:, cols], op=Alu.add)
                # fold into wrapped layout: rhs2 = blk (x) K, m2 = W16REP^T @ rhs2
                rhs2 = sp.tile([128, G, 8], f32, tag=f"rhs2{g}")
                b1, b2 = broadcast_tensor_aps(
                    blkf[:].rearrange("p (b u) -> p b u", u=1),
                    KM.rearrange("p (u j) -> p u j", u=1),
                )
                nc.vector.tensor_tensor(rhs2[:], b1, b2, op=Alu.mult)
                m2 = pp.tile([128, G * 8], f32)
                nc.tensor.matmul(
                    out=m2[:], lhsT=WM, rhs=rhs2[:].rearrange("p b j -> p (b j)"),
                    start=True, stop=True,
                )
                nc.vector.tensor_copy(AIDX[:, 8 * lo : 8 * hi], m2[:])
                nc.gpsimd.dma_gather(
                    ADJB[:, cols, :],
                    av[lo * ROWS_PER_B : hi * ROWS_PER_B],
                    AIDX[:, 8 * lo : 8 * hi],
                    G * 128, G * 128, AE, queue_num=GQ[g],
                )
                # fused select: Wv[:, b] = sum_k (iota==rem) * block
                scrA = sp.tile([128, AE], f32, tag=f"scrA{g}")
                for b in range(lo, hi):
                    nc.vector.scalar_tensor_tensor(
                        scrA[:], IO, remf[:, b : b + 1], ADJB[:, b, :],
                        op0=Alu.is_equal, op1=Alu.mult,
                        accum_out=Wv[:, b : b + 1],
                    )

            for b in range(2):
                with tc.tile_wait_until(0.002 * b):
                    batch(b)
            with tc.tile_wait_until(0.0045):
                adj_group(0)
            for b in range(2, 4):
                with tc.tile_wait_until(0.002 * b):
                    batch(b)
            with tc.tile_wait_until(0.0085):
                adj_group(1)
            for b in range(4, 6):
                with tc.tile_wait_until(0.002 * b):
                    batch(b)
            with tc.tile_wait_until(0.0125):
                adj_group(2)
            for b in range(6, BPC):
                with tc.tile_wait_until(0.002 * b):
                    batch(b)

            with tc.tile_wait_until(0.013):
                # fused XT selects (XTB ready long before; fills DVE gaps)
                scrX = sp.tile([128, XE], f32, tag="scrX")
                for b in range(BPC):
                    nc.vector.scalar_tensor_tensor(
                        scrX[:], IO[:, 0:XE], XR[:, b : b + 1], XTB[:, b, :],
                        op0=Alu.is_equal, op1=Alu.mult,
                        accum_out=XTv[:, b : b + 1],
                    )

            with tc.tile_wait_until(0.0165):
                adj_group(3)

            with tc.tile_wait_until(0.017):
                # cross-entropy partial
                LSE = accp.tile([L, BPC], f32)
                nc.scalar.activation(LSE[:], S[:], AF.Ln)
                R = accp.tile([L, 2], f32)
                nc.vector.memset(R[:, 1:2], 0.0)
                NLL = accp.tile([L, BPC], f32)
                nc.vector.tensor_sub(NLL[:], LSE[:], XTv[:])
                nc.vector.reduce_sum(R[:, 0:1], NLL[:], axis=AX.X)

            with tc.tile_wait_until(0.018):
                # balance partial: row 127 of Wv is pad
                ps_b = pp.tile([BPC, 1], f32)
                nc.tensor.matmul(
                    out=ps_b[:], lhsT=Wv[0 : L - 1, :], rhs=ones[0 : L - 1, :],
                    start=True, stop=True,
                )
                bneg = sp.tile([BPC, 1], f32, tag="bneg")
                nc.vector.tensor_scalar_mul(bneg[:], ps_b[:], -1.0)
                nc.vector.tensor_tensor(R[0:BPC, 1:2], ps_b[:], bneg[:], op=Alu.max)

                ps2 = pp.tile([2, 1], f32)
                nc.tensor.matmul(out=ps2[:], lhsT=R[:], rhs=ones[:], start=True, stop=True)
                c2 = sp.tile([2, 1], f32, tag="c2")
                nc.vector.tensor_copy(c2[:], ps2[:])
                nc.scalar.dma_start(out.ap(), c2[:])

    nc.compile()
    return nc


def _get_nc():
    if "nc" not in _CACHE:
        _CACHE["nc"] = _build()
    return _CACHE["nc"]


def _consts():
    if "consts" in _CACHE:
        return _CACHE["consts"]
    ls = np.arange(128)
    cfm = np.zeros((128, 344), np.float32)
    cfm[:, 8:16] = (ls[:, None] // 16 == np.arange(8)[None, :]).astype(np.float32)
    cfm[:, 16:144] = (ls[:, None] % 16 == ls[None, :] % 16).astype(np.float32)
    cfm[:, 144 : 144 + AE] = np.arange(AE, dtype=np.float32)[None, :]
    cfm[:, 208:336] = (ls[:, None] == ls[None, :] + 1).astype(np.float32)
    cc = np.zeros(BPC, np.float32)
    for lo, hi in GROUPS:
        cc[lo:hi] = (np.arange(hi - lo)) * (N * N // AE)
    cfm[:, 336:344] = cc[None, :]
    _CACHE["consts"] = cfm
    return _CACHE["consts"]


def make_in_maps(path_logits, target_paths, adj_matrix):
    """Shard full inputs into per-core in_maps (host-side packing only)."""
    cfm = _consts()
    l_arange = np.arange(L, dtype=np.int64)
    in_maps = []
    for c in range(NCORES):
        sl = slice(c * BPC, (c + 1) * BPC)
        lgc = np.ascontiguousarray(path_logits[sl], dtype=np.float32).reshape(
            BPC * L * N // XE, XE
        )
        adc = np.ascontiguousarray(adj_matrix[sl], dtype=np.float32).reshape(
            BPC * N * N // AE, AE
        )
        t = np.asarray(target_paths[sl], dtype=np.int64)  # [BPC, L]
        blkx = (
            np.arange(BPC, dtype=np.int64)[:, None] * (L * N // XE)
            + l_arange[None, :] * (N // XE)
            + (t >> 6)
        )
        xti = np.zeros((16, 64), np.int16)
        g = (np.arange(BPC)[:, None] * 128 + l_arange[None, :]).ravel()
        xti[g % 16, g // 16] = blkx.ravel()
        cum = np.tile(xti, (8, 1))
        cfc = cfm.copy()
        cfc[:, 0:BPC] = (t & (XE - 1)).T.astype(np.float32)
        in_maps.append({"logits": lgc, "adj": adc, "cf": cfc, "cu": cum})
    return in_maps


def kernel(**inputs):
    from concourse import bass_utils

    nc = _get_nc()
    in_maps = make_in_maps(
        inputs["path_logits"], inputs["target_paths"], inputs["adj_matrix"]
    )
    res = bass_utils.run_bass_kernel_spmd(nc, in_maps, core_ids=list(range(NCORES)))
    w_nll = np.float32((1.0 - ALPHA) / (B * L))
    w_bal = np.float32(ALPHA / B)
    total = np.float32(0.0)
    for r in res.results:
        total = total + w_nll * np.float32(r["out"][0, 0]) + w_bal * np.float32(
            r["out"][1, 0]
        )
    return np.asarray(total, dtype=np.float32)


# revision 20
# speedup vs baseline: 1.2677x; 1.2677x over previous
"""Trainium2 Bass kernel for CycleBalanceLoss.

loss = ALPHA * mean_b |sum_l adj[b, argmax_l, argmax_{l+1}]|
     + (1-ALPHA) * mean_{b,l} (logsumexp(logits[b,l,:]) - logits[b,l,t[b,l]])

Sharding: pure data parallel over the batch dim B=64 across 8 cores
(BPC=8 batches per core). Host sums the 8 per-core partial scalars.

Per core:
  - stream the logits shard [8, 128, 1024] f32 through SBUF;
  - ScalarE computes exp(x) with a per-row f32 accumulator (-> logsumexp),
    writing the exp values as fp16: argmax(exp(x)) == argmax(x), so the
    DVE max/max_index pass runs on 2-byte data;
  - target logits are gathered with per-column HW-DGE indirect DMAs
    (host-precomputed flat indices) issued before the loop so they overlap
    the stream;
  - the adjacency pair index pair[l] = 1024*idx[l] + idx[l+1] is ONE tiny
    PE matmul per batch against PAIRM = 1024*I + subdiag(1) (the subdiag
    also does the l+1 partition shift and zeroes the pad row), then a
    cast to u32 feeds a per-batch indirect DMA issued right after that
    batch's argmax, so gathers pace with the loop and only the last
    batch's gather (~2.7us + transfer) sits in the tail;
  - tc.tile_wait_until phases pin per-engine queue order so the tile
    scheduler cannot hoist gather-dependent ops ahead of the argmax work
    (its DMA cost model is optimistic, which otherwise stalls the
    in-order queues).
"""

import numpy as np

B, L, N = 64, 128, 1024
NCORES = 8
BPC = B // NCORES
ALPHA = 0.7

_CACHE = {}


def _build():
    import concourse.bacc as bacc
    import concourse.tile as tile
    from concourse import bass, mybir

    f32 = mybir.dt.float32
    fp16 = mybir.dt.float16
    i32 = mybir.dt.int32
    u16 = mybir.dt.uint16
    u32 = mybir.dt.uint32
    AF = mybir.ActivationFunctionType
    Alu = mybir.AluOpType
    AX = mybir.AxisListType

    nc = bacc.Bacc(
        "TRN2",
        target_bir_lowering=False,
        debug=False,
        num_devices=NCORES,
    )

    logits = nc.dram_tensor("logits", [BPC, L, N], f32, kind="ExternalInput")
    tfidx = nc.dram_tensor("tfidx", [L, BPC], i32, kind="ExternalInput")
    adj = nc.dram_tensor("adj", [BPC * N * N, 1], f32, kind="ExternalInput")
    pairm = nc.dram_tensor("pairm", [128, 128], f32, kind="ExternalInput")
    out = nc.dram_tensor("out", [2, 1], f32, kind="ExternalOutput")

    logits_ap = logits.ap()
    logits_flat = logits_ap.rearrange("b l n -> (b l n)")[:, None]

    with tile.TileContext(nc) as tc:
        with (
            tc.tile_pool(name="xp", bufs=4) as xp,
            tc.tile_pool(name="ep", bufs=3) as ep,
            tc.tile_pool(name="sp", bufs=3) as sp,
            tc.tile_pool(name="acc", bufs=1) as accp,
            tc.tile_pool(name="psum", bufs=2, space="PSUM") as pp,
        ):
            ones = accp.tile([L, 1], f32)
            nc.vector.memset(ones[:], 1.0)

            PM = accp.tile([128, 128], f32)
            nc.scalar.dma_start(PM[:], pairm.ap())

            # target flat indices (host precomputed) and target-logit gather.
            # NOTE: multi-column offset tables wedge the HW DGE - one
            # indirect DMA per column ([P,1] offsets) is the proven shape.
            TF = accp.tile([L, BPC], i32)
            nc.scalar.dma_start(TF[:], tfidx.ap())
            XT = accp.tile([L, BPC], f32)
            for b in range(BPC):
                nc.gpsimd.indirect_dma_start(
                    out=XT[:, b : b + 1],
                    out_offset=None,
                    in_=logits_flat,
                    in_offset=bass.IndirectOffsetOnAxis(ap=TF[:, b : b + 1], axis=0),
                )

            S = accp.tile([L, BPC], f32)
            M8 = accp.tile([L, BPC * 8], fp16)
            W = accp.tile([L, BPC], f32)  # rows 0..126 hold path weights

            def batch(b):
                X = xp.tile([L, N], f32, tag="X")
                nc.sync.dma_start(X[:], logits_ap[b])
                E = ep.tile([L, N], fp16, tag="E")
                nc.scalar.activation(E[:], X[:], AF.Exp, accum_out=S[:, b : b + 1])
                nc.vector.max(M8[:, 8 * b : 8 * b + 8], E[:])
                i8 = sp.tile([L, 8], u16, tag="i8")
                nc.vector.max_index(i8[:], M8[:, 8 * b : 8 * b + 8], E[:])

                # pair[l] = 1024*idx[l] + idx[l+1] via one PE matmul
                idxf = sp.tile([L, 1], f32, tag="idxf")
                nc.vector.tensor_copy(idxf[:], i8[:, 0:1])
                pairp = pp.tile([L, 1], f32)
                nc.tensor.matmul(
                    out=pairp[:], lhsT=PM[:], rhs=idxf[:], start=True, stop=True
                )
                pairu = sp.tile([L, 1], u32, tag="pairu")
                nc.vector.tensor_copy(pairu[0 : L - 1, :], pairp[0 : L - 1, :])
                nc.gpsimd.indirect_dma_start(
                    out=W[0 : L - 1, b : b + 1],
                    out_offset=None,
                    in_=adj.ap(),
                    in_offset=bass.IndirectOffsetOnAxis(
                        ap=pairu[0 : L - 1, :], axis=0
                    ),
                    element_offset=b * N * N,
                )

            for b in range(BPC):
                with tc.tile_wait_until(0.0024 * b):
                    batch(b)

            with tc.tile_wait_until(0.0195):
                # cross-entropy partial: sum(ln S - x_t)
                LSE = accp.tile([L, BPC], f32)
                nc.scalar.activation(LSE[:], S[:], AF.Ln)
                R = accp.tile([L, 2], f32)
                nc.vector.memset(R[:, 1:2], 0.0)
                NLL = accp.tile([L, BPC], f32)
                nc.vector.tensor_sub(NLL[:], LSE[:], XT[:])
                nc.vector.reduce_sum(R[:, 0:1], NLL[:], axis=AX.X)

            with tc.tile_wait_until(0.021):
                # balance partial: |per-batch path sums| via PE
                ps_b = pp.tile([BPC, 1], f32)
                nc.tensor.matmul(
                    out=ps_b[:], lhsT=W[0 : L - 1, :], rhs=ones[0 : L - 1, :],
                    start=True, stop=True,
                )
                bneg = sp.tile([BPC, 1], f32, tag="bneg")
                nc.vector.tensor_scalar_mul(bneg[:], ps_b[:], -1.0)
                nc.vector.tensor_tensor(R[0:BPC, 1:2], ps_b[:], bneg[:], op=Alu.max)

                ps2 = pp.tile([2, 1], f32)
                nc.tensor.matmul(out=ps2[:], lhsT=R[:], rhs=ones[:], start=True, stop=True)
                c2 = sp.tile([2, 1], f32, tag="c2")
                nc.vector.tensor_copy(c2[:], ps2[:])
                nc.scalar.dma_start(out.ap(), c2[:])

    nc.compile()
    return nc


def _get_nc():
    if "nc" not in _CACHE:
        _CACHE["nc"] = _build()
    return _CACHE["nc"]


def _consts():
    if "consts" in _CACHE:
        return _CACHE["consts"]
    ls = np.arange(128)
    pairmm = 1024.0 * (ls[:, None] == ls[None, :]).astype(np.float32) + (
        ls[:, None] == ls[None, :] + 1
    ).astype(np.float32)
    _CACHE["consts"] = pairmm
    return _CACHE["consts"]


def make_in_maps(path_logits, target_paths, adj_matrix):
    """Shard full inputs into per-core in_maps (host-side packing only)."""
    pairmm = _consts()
    l_off = np.arange(L, dtype=np.int64) * N
    b_off = np.arange(BPC, dtype=np.int64)[:, None] * (L * N)
    in_maps = []
    for c in range(NCORES):
        sl = slice(c * BPC, (c + 1) * BPC)
        lg = np.ascontiguousarray(path_logits[sl], dtype=np.float32)
        ad = np.ascontiguousarray(adj_matrix[sl], dtype=np.float32).reshape(
            BPC * N * N, 1
        )
        t = np.asarray(target_paths[sl], dtype=np.int64)
        tf = (b_off + l_off[None, :] + t).astype(np.int32)
        in_maps.append(
            {
                "logits": lg,
                "tfidx": np.ascontiguousarray(tf.T),
                "adj": ad,
                "pairm": pairmm,
            }
        )
    return in_maps


def kernel(**inputs):
    from concourse import bass_utils

    nc = _get_nc()
    in_maps = make_in_maps(
        inputs["path_logits"], inputs["target_paths"], inputs["adj_matrix"]
    )
    res = bass_utils.run_bass_kernel_spmd(nc, in_maps, core_ids=list(range(NCORES)))
    w_nll = np.float32((1.0 - ALPHA) / (B * L))
    w_bal = np.float32(ALPHA / B)
    total = np.float32(0.0)
    for r in res.results:
        total = total + w_nll * np.float32(r["out"][0, 0]) + w_bal * np.float32(
            r["out"][1, 0]
        )
    return np.asarray(total, dtype=np.float32)
